# revision 1
# baseline (speedup 1.0000x reference)
"""Trainium2 Bass kernel for nn_CriticUAVob (attention-pool critic).

Math: for each batch item b (4096 total), two attention-pool branches over
s_b [N=128, 3], then a small MLP.  Key identity used: with P = softmax(S)
row-wise and V = s'Wv',

    mean_n (P V)[n] = (1/N) * c^T s' Wv',   c[m] = sum_n U[n,m] / Z[n]

so V is never materialized.  Per item we compute S^T = s' A~ s'^T (K=4
matmuls), U^T = exp(S^T) on ScalarE, G = U^T^T-weighted sums of s' (one
K=128 matmul whose ones-column yields Z), r = 1/Z, t = G^T r (tiny matmul),
and finally a batched MLP over all items at the end.

Sharding: pure data parallel, batch split across 8 NeuronCores.
"""
import os
import sys
import numpy as np

sys.path.insert(0, "/opt/trn_rl_repo")

import concourse.bass as bass
import concourse.tile as tile
from concourse import bacc, mybir
from concourse import bass_utils
from concourse.masks import make_identity

N_CORES = 8
B = 4096
N = 128
BC = B // N_CORES          # 512 items per core
QUADS = BC // 4            # 128 groups of 4 items
F32 = mybir.dt.float32
AF = mybir.ActivationFunctionType

_cache = {}


def _build():
    nc = bacc.Bacc(
        "TRN2",
        target_bir_lowering=False,
        debug=False,
        enable_asserts=False,
        num_devices=N_CORES,
    )
    s_t = nc.dram_tensor("s", [BC, N, 3], F32, kind="ExternalInput")
    amat_t = nc.dram_tensor("amat", [4, 8], F32, kind="ExternalInput")
    wcrs_t = nc.dram_tensor("wcrs", [4, 64], F32, kind="ExternalInput")
    wctg_t = nc.dram_tensor("wctg", [4, 64], F32, kind="ExternalInput")
    w1_t = nc.dram_tensor("w1", [64, 128], F32, kind="ExternalInput")
    w2_t = nc.dram_tensor("w2", [128, 128], F32, kind="ExternalInput")
    w3_t = nc.dram_tensor("w3", [128, 1], F32, kind="ExternalInput")
    b1_t = nc.dram_tensor("b1", [128, 1], F32, kind="ExternalInput")
    b2_t = nc.dram_tensor("b2", [128, 1], F32, kind="ExternalInput")
    b3_t = nc.dram_tensor("b3rep", [1, BC], F32, kind="ExternalInput")
    out_t = nc.dram_tensor("out", [BC, 1], F32, kind="ExternalOutput")

    s_ap = s_t.ap()

    with tile.TileContext(nc) as tc:
        with (
            tc.tile_pool(name="singles", bufs=1) as singles,
            tc.tile_pool(name="qsb", bufs=3) as qsb,
            tc.tile_pool(name="pst", bufs=4, space="PSUM") as pst,
            tc.tile_pool(name="psmall", bufs=3, space="PSUM") as psmall,
        ):
            ident = singles.tile([128, 128], F32)
            make_identity(nc, ident[:])
            amat = singles.tile([4, 8], F32)
            nc.sync.dma_start(amat[:], amat_t.ap())
            wcrs = singles.tile([4, 64], F32)
            nc.sync.dma_start(wcrs[:], wcrs_t.ap())
            wctg = singles.tile([4, 64], F32)
            nc.sync.dma_start(wctg[:], wctg_t.ap())
            w1 = singles.tile([64, 128], F32)
            nc.sync.dma_start(w1[:], w1_t.ap())
            w2 = singles.tile([128, 128], F32)
            nc.sync.dma_start(w2[:], w2_t.ap())
            w3 = singles.tile([128, 1], F32)
            nc.sync.dma_start(w3[:], w3_t.ap())
            b1 = singles.tile([128, 1], F32)
            nc.sync.dma_start(b1[:], b1_t.ap())
            b2 = singles.tile([128, 1], F32)
            nc.sync.dma_start(b2[:], b2_t.ap())
            b3r = singles.tile([1, BC], F32)
            nc.sync.dma_start(b3r[:], b3_t.ap())
            # T^T accumulator: rows k=0..3, cols = item*2 + branch
            tbig = singles.tile([4, 2 * BC], F32)

            for q in range(QUADS):
                # ---- load 4 items' s as [n, (item, k)] with a ones column
                s_nat = qsb.tile([128, 16], F32, tag="s_nat")
                src = s_ap[q * 4:(q + 1) * 4].rearrange("i n k -> n i k")
                dst = s_nat[:].rearrange("n (i f) -> n i f", i=4)
                nc.sync.dma_start(dst[:, :, 0:3], src)
                nc.gpsimd.memset(dst[:, :, 3:4], 1.0)

                # ---- transpose each item: sT[k, n] (4 rows incl ones row)
                ps_t = psmall.tile([4, 512], F32, tag="ps_sm")
                for i in range(4):
                    nc.tensor.transpose(
                        ps_t[:, i * 128:(i + 1) * 128],
                        s_nat[:, i * 4:(i + 1) * 4],
                        ident[:],
                    )
                sT = qsb.tile([4, 512], F32, tag="sT")
                nc.vector.tensor_copy(sT[:], ps_t[:])

                # ---- BT = A~ s'^T per branch (PSUM reads must be 32-aligned,
                # so two [4,512] tiles rather than one [8,512])
                ps_btr = psmall.tile([4, 512], F32, tag="ps_sm")
                ps_btt = psmall.tile([4, 512], F32, tag="ps_sm")
                nc.tensor.matmul(ps_btr[:], amat[:, 0:4], sT[:])
                nc.tensor.matmul(ps_btt[:], amat[:, 4:8], sT[:])
                bt_rs = qsb.tile([4, 512], F32, tag="bt_rs")
                bt_tg = qsb.tile([4, 512], F32, tag="bt_tg")
                nc.vector.tensor_copy(bt_rs[:], ps_btr[:])
                nc.vector.tensor_copy(bt_tg[:], ps_btt[:])

                # ---- S^T per item per branch, then exp
                st_rs = pst.tile([128, 512], F32, tag="st")
                st_tg = pst.tile([128, 512], F32, tag="st")
                for i in range(4):
                    sl = slice(i * 128, (i + 1) * 128)
                    nc.tensor.matmul(st_rs[:, sl], sT[:, sl], bt_rs[:, sl])
                    nc.tensor.matmul(st_tg[:, sl], sT[:, sl], bt_tg[:, sl])
                ut_rs = qsb.tile([128, 512], F32, tag="ut_rs")
                ut_tg = qsb.tile([128, 512], F32, tag="ut_tg")
                nc.scalar.activation(ut_rs[:], st_rs[:], AF.Exp)
                nc.scalar.activation(ut_tg[:], st_tg[:], AF.Exp)

                # ---- G = sum_m U^T[m,n] * s'[m,k]  -> [n, 4]; col 3 = Z
                ps_g = psmall.tile([128, 32], F32, tag="ps_sm")
                for i in range(4):
                    sl = slice(i * 128, (i + 1) * 128)
                    nsl = slice(i * 4, (i + 1) * 4)
                    c0 = (i * 2) * 4
                    c1 = (i * 2 + 1) * 4
                    nc.tensor.matmul(ps_g[:, c0:c0 + 4], ut_rs[:, sl], s_nat[:, nsl])
                    nc.tensor.matmul(ps_g[:, c1:c1 + 4], ut_tg[:, sl], s_nat[:, nsl])
                g_sb = qsb.tile([128, 32], F32, tag="g_sb")
                nc.vector.tensor_copy(g_sb[:], ps_g[:])
                r_sb = qsb.tile([128, 8], F32, tag="r_sb")
                g3 = g_sb[:].rearrange("n (c f) -> n c f", f=4)
                nc.vector.reciprocal(r_sb[:], g3[:, :, 3])

                # ---- t = G^T r  -> [4, 1] per (item, branch)
                ps_tt = psmall.tile([4, 8], F32, tag="ps_sm")
                for c in range(8):
                    nc.tensor.matmul(
                        ps_tt[:, c:c + 1],
                        g_sb[:, c * 4:(c + 1) * 4],
                        r_sb[:, c:c + 1],
                    )
                nc.vector.tensor_copy(tbig[:, q * 8:(q + 1) * 8], ps_tt[:])

            # ---- batched MLP over all BC items
            tb3 = tbig[:].rearrange("p (b j) -> p j b", j=2)
            ps_h = pst.tile([64, BC], F32, tag="st")
            nc.tensor.matmul(ps_h[:], wcrs[:], tb3[:, 0, :], start=True, stop=False)
            nc.tensor.matmul(ps_h[:], wctg[:], tb3[:, 1, :], start=False, stop=True)
            h_sb = singles.tile([64, BC], F32)
            nc.vector.tensor_copy(h_sb[:], ps_h[:])

            ps_z1 = pst.tile([128, BC], F32, tag="st")
            nc.tensor.matmul(ps_z1[:], w1[:], h_sb[:])
            h1 = singles.tile([128, BC], F32)
            nc.scalar.activation(h1[:], ps_z1[:], AF.Tanh, bias=b1[:])

            ps_z2 = pst.tile([128, BC], F32, tag="st")
            nc.tensor.matmul(ps_z2[:], w2[:], h1[:])
            h2 = singles.tile([128, BC], F32)
            nc.scalar.activation(h2[:], ps_z2[:], AF.Tanh, bias=b2[:])

            ps_z3 = psmall.tile([1, BC], F32, tag="ps_sm")
            nc.tensor.matmul(ps_z3[:], w3[:], h2[:])
            y_sb = singles.tile([1, BC], F32)
            nc.vector.tensor_add(y_sb[:], ps_z3[:], b3r[:])

            nc.sync.dma_start(out_t.ap().rearrange("b o -> o b"), y_sb[:])

    nc.compile()
    return nc


def _host_prep(inputs):
    f = lambda x: np.asarray(x, dtype=np.float32)
    s_obs = f(inputs["s_obs"])

    def aug(W, b):
        return np.vstack([f(W), f(b).reshape(1, -1)])  # [4, dout]

    Wq_rs = aug(inputs["Wq_rs"], inputs["bq_rs"])
    Wk_rs = aug(inputs["Wk_rs"], inputs["bk_rs"])
    Wv_rs = aug(inputs["Wv_rs"], inputs["bv_rs"])
    Wq_tg = aug(inputs["Wq_tg"], inputs["bq_tg"])
    Wk_tg = aug(inputs["Wk_tg"], inputs["bk_tg"])
    Wv_tg = aug(inputs["Wv_tg"], inputs["bv_tg"])

    scale = 1.0 / np.sqrt(16.0)
    # S^T orientation needs A~ = A^T where A = Wq' Wk'^T * scale
    At_rs = (Wq_rs @ Wk_rs.T * scale).T.astype(np.float32)
    At_tg = (Wq_tg @ Wk_tg.T * scale).T.astype(np.float32)
    amat = np.concatenate([At_rs.T, At_tg.T], axis=1).astype(np.float32)  # [4,8]

    wcrs = np.zeros((4, 64), np.float32)
    wctg = np.zeros((4, 64), np.float32)
    wcrs[:, 0:32] = Wv_rs / N
    wctg[:, 32:64] = Wv_tg / N

    w1 = f(inputs["W1"])                       # [64, 128]
    b1 = f(inputs["b1"]).reshape(128, 1)
    w2 = f(inputs["W2"])                       # [128, 128]
    b2 = f(inputs["b2"]).reshape(128, 1)
    w3 = f(inputs["W3"])                       # [128, 1]
    b3rep = np.full((1, BC), float(np.asarray(inputs["b3"]).reshape(-1)[0]),
                    np.float32)

    common = dict(amat=amat, wcrs=wcrs, wctg=wctg, w1=w1, w2=w2, w3=w3,
                  b1=b1, b2=b2, b3rep=b3rep)
    in_maps = []
    for c in range(N_CORES):
        m = dict(common)
        m["s"] = np.ascontiguousarray(s_obs[c * BC:(c + 1) * BC])
        in_maps.append(m)
    return in_maps


def kernel(**inputs):
    if "nc" not in _cache:
        _cache["nc"] = _build()
    nc = _cache["nc"]
    in_maps = _host_prep(inputs)
    trace = os.environ.get("KERNEL_TRACE", "0") == "1"
    res = bass_utils.run_bass_kernel_spmd(
        nc, in_maps, core_ids=list(range(N_CORES)), trace=trace
    )
    _cache["last"] = res
    out = np.concatenate([r["out"] for r in res.results], axis=0)
    return out.astype(np.float32)



# revision 12
# speedup vs baseline: 3.1510x; 3.1510x over previous
"""Trainium2 Bass kernel for nn_CriticUAVob (attention-pool critic).

Math per item b (4096 total): two attention-pool branches over s_b [N=128, 3]
followed by a small MLP.  With s' = [s, 1] (N x 4) and A = Wq' Wk'^T / 4:

    S = s' A s'^T,  U = exp(S),  Z[n] = sum_m U[n,m]
    pooled = (1/N) * sum_n (U[n,:] / Z[n]) @ V,   V = s' Wv'
           = (1/N) * t^T Wv',   t[k] = sum_n (sum_m U[n,m] s'[m,k]) / Z[n]

Device pipeline (per quad of 4 items, batch data-parallel over 8 cores):
  - one DMA brings Y = A^T s'^T per item in a block-diagonal layout (qt) plus
    s'^T stacked (sst); a second DMA brings zero-padded s' variants (snatp)
  - 2 matmuls (lhsT=sst[16,128], rhs=qt[16,512]) -> X = S^T per item, both
    branches; the block-diagonal qt kills cross-item terms
  - 2 ScalarE exp -> U^T in bf16
  - 4 accumulating matmuls (lhsT = zero-padded s'_i [128,16], rhs = U^T item
    cols) -> G[(i,k),(b,n)] with no cross-item garbage; s' ones-column makes
    row (i,3) = Z
  - tiny const matmul replicates each Z row over its item's 4 rows; vector
    reciprocal + 2 fused multiply-reduce (tensor_tensor_reduce) produce
    t[(i,k)] per branch straight into an accumulator tile
  - batched MLP over all 512 items at the end

All PE inputs are bf16 (4x matmul throughput vs fp32, half the LDWEIGHTS).
"""
import os
import sys
import numpy as np
import ml_dtypes

sys.path.insert(0, "/opt/trn_rl_repo")

import concourse.bass as bass
import concourse.tile as tile
from concourse import bacc, mybir
from concourse import bass_utils

N_CORES = 8
B = 4096
N = 128
BC = B // N_CORES          # 512 items per core
QUADS = BC // 4            # 128 groups of 4 items
F32 = mybir.dt.float32
BF16 = mybir.dt.bfloat16
AF = mybir.ActivationFunctionType
ALU = mybir.AluOpType
BF = ml_dtypes.bfloat16

_cache = {}


def _build():
    nc = bacc.Bacc(
        "TRN2",
        target_bir_lowering=False,
        debug=False,
        enable_asserts=False,
        num_devices=N_CORES,
    )
    # per-quad data: qt [16, 1024] block-diag A^T s'^T (both branches) then
    # sst [16, 128] = stacked s'^T, packed in one contiguous record
    qtss_t = nc.dram_tensor("qtss", [QUADS, 16, 1152], BF16, kind="ExternalInput")
    # zero-padded s' variants: cols (i, k16); variant i nonzero only in 4i..4i+4
    snatp_t = nc.dram_tensor("snatp", [QUADS, 128, 64], BF16, kind="ExternalInput")
    rep16_t = nc.dram_tensor("rep16", [16, 16], BF16, kind="ExternalInput")
    wcrs_t = nc.dram_tensor("wcrs", [4, 64], F32, kind="ExternalInput")
    wctg_t = nc.dram_tensor("wctg", [4, 64], F32, kind="ExternalInput")
    w1_t = nc.dram_tensor("w1", [64, 128], BF16, kind="ExternalInput")
    w2_t = nc.dram_tensor("w2", [128, 128], BF16, kind="ExternalInput")
    w3_t = nc.dram_tensor("w3", [128, 1], BF16, kind="ExternalInput")
    b1_t = nc.dram_tensor("b1", [128, 1], F32, kind="ExternalInput")
    b2_t = nc.dram_tensor("b2", [128, 1], F32, kind="ExternalInput")
    b3_t = nc.dram_tensor("b3rep", [1, BC], F32, kind="ExternalInput")
    out_t = nc.dram_tensor("out", [BC, 1], F32, kind="ExternalOutput")

    qtss_ap = qtss_t.ap()
    snatp_ap = snatp_t.ap()

    with tile.TileContext(nc) as tc:
        with (
            tc.tile_pool(name="singles", bufs=1) as singles,
            tc.tile_pool(name="qsb", bufs=3) as qsb,
            tc.tile_pool(name="pst", bufs=4, space="PSUM") as pst,
            tc.tile_pool(name="psmall", bufs=2, space="PSUM") as psmall,
        ):
            rep16 = singles.tile([16, 16], BF16)
            nc.sync.dma_start(rep16[:], rep16_t.ap())
            wcrs = singles.tile([4, 64], F32)
            nc.sync.dma_start(wcrs[:], wcrs_t.ap())
            wctg = singles.tile([4, 64], F32)
            nc.sync.dma_start(wctg[:], wctg_t.ap())
            w1 = singles.tile([64, 128], BF16)
            nc.sync.dma_start(w1[:], w1_t.ap())
            w2 = singles.tile([128, 128], BF16)
            nc.sync.dma_start(w2[:], w2_t.ap())
            w3 = singles.tile([128, 1], BF16)
            nc.sync.dma_start(w3[:], w3_t.ap())
            b1 = singles.tile([128, 1], F32)
            nc.sync.dma_start(b1[:], b1_t.ap())
            b2 = singles.tile([128, 1], F32)
            nc.sync.dma_start(b2[:], b2_t.ap())
            b3r = singles.tile([1, BC], F32)
            nc.sync.dma_start(b3r[:], b3_t.ap())
            # t accumulator: rows (i,k), cols (quad, branch)
            tbig = singles.tile([16, 2 * QUADS], F32)

            for q in range(QUADS):
                qtss = qsb.tile([16, 1152], BF16, tag="qtss")
                nc.sync.dma_start(qtss[:], qtss_ap[q])
                snatp = qsb.tile([128, 64], BF16, tag="snatp")
                nc.gpsimd.dma_start(snatp[:], snatp_ap[q])
                sst = qtss[:, 1024:1152]

                # X = S^T per item (key idx on partitions), both branches
                ps_st_rs = pst.tile([128, 512], F32, tag="st")
                ps_st_tg = pst.tile([128, 512], F32, tag="st")
                nc.tensor.matmul(ps_st_rs[:], sst, qtss[:, 0:512])
                nc.tensor.matmul(ps_st_tg[:], sst, qtss[:, 512:1024])

                # U^T = exp(X), bf16; cols (b, i, n)
                ut = qsb.tile([128, 1024], BF16, tag="ut")
                nc.scalar.activation(ut[:, 0:512], ps_st_rs[:], AF.Exp)
                nc.scalar.activation(ut[:, 512:1024], ps_st_tg[:], AF.Exp)

                # G[(i,k), (b,n)] = sum_m s'_i[m,k] U^T[m,(b,n)]; row (i,3)=Z
                ps_g = psmall.tile([16, 256], F32, tag="ps_g")
                ut3 = ut[:].rearrange("m (b i n) -> m b i n", b=2, i=4)
                for i in range(4):
                    nc.tensor.matmul(
                        ps_g[:],
                        snatp[:, 16 * i:16 * (i + 1)],
                        ut3[:, :, i, :],
                        start=(i == 0),
                        stop=(i == 3),
                    )

                # replicate each item's Z row over its 4 rows, then 1/Z
                g_sb = qsb.tile([16, 256], BF16, tag="g_sb")
                nc.vector.tensor_copy(g_sb[:], ps_g[:])
                ps_zrep = psmall.tile([16, 256], F32, tag="ps_zrep")
                nc.tensor.matmul(ps_zrep[:], rep16[:], g_sb[:])
                r_sb = qsb.tile([16, 256], F32, tag="r_sb")
                nc.vector.reciprocal(r_sb[:], ps_zrep[:])

                # t[(i,k), (q,b)] = sum_n G * (1/Z)
                pg = qsb.tile([16, 256], F32, tag="pg")
                nc.vector.tensor_mul(pg[:], ps_g[:], r_sb[:])
                pg3 = pg[:].rearrange("p (b n) -> p b n", b=2)
                nc.vector.tensor_reduce(
                    tbig[:, 2 * q:2 * (q + 1)], pg3,
                    axis=mybir.AxisListType.X, op=ALU.add,
                )

            # repack t to [4, (b, q, i)] = [4(k), (b, item)] via sbuf-to-sbuf
            # DMA (engine APs cannot start at partition 4, DMA descriptors can)
            trs = singles.tile([4, 2 * BC], F32)
            trs4 = trs[:].rearrange("p (b q i) -> p b q i", b=2, i=4)
            for i in range(4):
                src = tbig[4 * i:4 * (i + 1), :].rearrange("p (q b) -> p b q", b=2)
                for b in range(2):
                    nc.sync.dma_start(trs4[:, b, :, i], src[:, b, :])

            # batched MLP over all BC items
            ps_h = pst.tile([64, BC], F32, tag="st")
            nc.tensor.matmul(ps_h[:], wcrs[:], trs[:, 0:BC], start=True, stop=False)
            nc.tensor.matmul(ps_h[:], wctg[:], trs[:, BC:2 * BC], start=False, stop=True)
            h_sb = singles.tile([64, BC], BF16)
            nc.vector.tensor_copy(h_sb[:], ps_h[:])

            ps_z1 = pst.tile([128, BC], F32, tag="st")
            nc.tensor.matmul(ps_z1[:], w1[:], h_sb[:])
            h1 = singles.tile([128, BC], BF16)
            nc.scalar.activation(h1[:], ps_z1[:], AF.Tanh, bias=b1[:])

            ps_z2 = pst.tile([128, BC], F32, tag="st")
            nc.tensor.matmul(ps_z2[:], w2[:], h1[:])
            h2 = singles.tile([128, BC], BF16)
            nc.scalar.activation(h2[:], ps_z2[:], AF.Tanh, bias=b2[:])

            ps_z3 = psmall.tile([1, BC], F32, tag="ps_g")
            nc.tensor.matmul(ps_z3[:], w3[:], h2[:])
            y_sb = singles.tile([1, BC], F32)
            nc.vector.tensor_add(y_sb[:], ps_z3[:], b3r[:])

            nc.sync.dma_start(out_t.ap().rearrange("b o -> o b"), y_sb[:])

    nc.compile()
    return nc


def _host_prep(inputs):
    f = lambda x: np.asarray(x, dtype=np.float32)
    s_obs = f(inputs["s_obs"])

    def aug_w(W, b):
        return np.vstack([f(W), f(b).reshape(1, -1)])  # [4, dout]

    Wq_rs = aug_w(inputs["Wq_rs"], inputs["bq_rs"])
    Wk_rs = aug_w(inputs["Wk_rs"], inputs["bk_rs"])
    Wv_rs = aug_w(inputs["Wv_rs"], inputs["bv_rs"])
    Wq_tg = aug_w(inputs["Wq_tg"], inputs["bq_tg"])
    Wk_tg = aug_w(inputs["Wk_tg"], inputs["bk_tg"])
    Wv_tg = aug_w(inputs["Wv_tg"], inputs["bv_tg"])

    scale = 1.0 / np.sqrt(16.0)
    A_rs = (Wq_rs @ Wk_rs.T * scale).astype(np.float32)   # [4, 4]
    A_tg = (Wq_tg @ Wk_tg.T * scale).astype(np.float32)

    ones = np.ones((B, N, 1), np.float32)
    s_aug = np.concatenate([s_obs, ones], axis=2)          # [B, 128, 4]

    # Y_b[item] = A_b^T s'^T : [B, 4, 128]
    Y = np.stack([
        np.einsum("kj,ink->ijn", A_rs, s_aug),
        np.einsum("kj,ink->ijn", A_tg, s_aug),
    ], axis=0).astype(np.float32)                          # [2, B, 4, 128]

    rep16 = np.zeros((16, 16), BF)
    for i in range(4):
        rep16[4 * i + 3, 4 * i:4 * (i + 1)] = 1.0

    wcrs = np.zeros((4, 64), np.float32)
    wctg = np.zeros((4, 64), np.float32)
    wcrs[:, 0:32] = Wv_rs / N
    wctg[:, 32:64] = Wv_tg / N

    w1 = f(inputs["W1"])                       # [64, 128]
    b1 = f(inputs["b1"]).reshape(128, 1)
    w2 = f(inputs["W2"])                       # [128, 128]
    b2 = f(inputs["b2"]).reshape(128, 1)
    w3 = f(inputs["W3"])                       # [128, 1]
    b3rep = np.full((1, BC), float(np.asarray(inputs["b3"]).reshape(-1)[0]),
                    np.float32)

    common = dict(
        rep16=rep16,
        wcrs=wcrs, wctg=wctg,
        w1=w1.astype(BF), w2=w2.astype(BF), w3=w3.astype(BF),
        b1=b1, b2=b2, b3rep=b3rep,
    )

    in_maps = []
    for c in range(N_CORES):
        lo, hi = c * BC, (c + 1) * BC
        sa = s_aug[lo:hi].reshape(QUADS, 4, N, 4)          # [Q, i, n, k]
        Yc = Y[:, lo:hi].reshape(2, QUADS, 4, 4, N)        # [b, Q, i, j, n]

        # qt [Q, (i,j)=16, (b,i',n)=1024], block-diagonal in (i, i')
        qt = np.zeros((QUADS, 4, 4, 2, 4, N), np.float32)  # q i j b i' n
        for i in range(4):
            qt[:, i, :, 0, i, :] = Yc[0, :, i]
            qt[:, i, :, 1, i, :] = Yc[1, :, i]
        qt = qt.reshape(QUADS, 16, 1024)

        # sst [Q, (i,k)=16, n=128]
        sst = sa.transpose(0, 1, 3, 2).reshape(QUADS, 16, N)

        qtss = np.concatenate([qt, sst], axis=2).astype(BF)  # [Q, 16, 1152]

        # snatp [Q, 128, (i, k16)=64]: variant i nonzero only in cols 4i..4i+4
        snatp = np.zeros((QUADS, N, 4, 16), np.float32)
        for i in range(4):
            snatp[:, :, i, 4 * i:4 * (i + 1)] = sa[:, i]
        snatp = snatp.reshape(QUADS, N, 64).astype(BF)

        m = dict(common)
        m["qtss"] = np.ascontiguousarray(qtss)
        m["snatp"] = np.ascontiguousarray(snatp)
        in_maps.append(m)
    return in_maps


def kernel(**inputs):
    if "nc" not in _cache:
        _cache["nc"] = _build()
    nc = _cache["nc"]
    in_maps = _host_prep(inputs)
    trace = os.environ.get("KERNEL_TRACE", "0") == "1"
    res = bass_utils.run_bass_kernel_spmd(
        nc, in_maps, core_ids=list(range(N_CORES)), trace=trace
    )
    _cache["last"] = res
    out = np.concatenate([r["out"] for r in res.results], axis=0)
    return out.astype(np.float32)


# revision 15
# speedup vs baseline: 4.5983x; 1.4593x over previous
"""Trainium2 Bass kernel for nn_CriticUAVob (attention-pool critic).

Math per item b (4096 total): two attention-pool branches over s_b [N=128, 3]
followed by a small MLP.  With s' = [s, 1] (N x 4) and A = Wq' Wk'^T / 4:

    S = s' A s'^T,  U = exp(S),  Z[n] = sum_m U[n,m]
    pooled = (1/N) * sum_n (U[n,:] / Z[n]) @ V,   V = s' Wv'
           = (1/N) * t^T Wv',   t[k] = sum_n (sum_m U[n,m] s'[m,k]) / Z[n]

Device pipeline (per quad of 4 items, batch data-parallel over 8 cores):
  - one DMA brings Y = A^T s'^T per item in a block-diagonal layout (qt) plus
    s'^T stacked (sst); a second DMA brings zero-padded s' variants (snatp)
  - 2 matmuls (lhsT=sst[16,128], rhs=qt[16,512]) -> X = S^T per item, both
    branches; the block-diagonal qt kills cross-item terms
  - 2 ScalarE exp -> U^T in bf16
  - 4 accumulating matmuls (lhsT = zero-padded s'_i [128,16], rhs = U^T item
    cols) -> G[(i,k),(b,n)] with no cross-item garbage; s' ones-column makes
    row (i,3) = Z
  - tiny const matmul replicates each Z row over its item's 4 rows; vector
    reciprocal + 2 fused multiply-reduce (tensor_tensor_reduce) produce
    t[(i,k)] per branch straight into an accumulator tile
  - batched MLP over all 512 items at the end

All PE inputs are bf16 (4x matmul throughput vs fp32, half the LDWEIGHTS).
"""
import os
import sys
import numpy as np
import ml_dtypes

sys.path.insert(0, "/opt/trn_rl_repo")

import concourse.bass as bass
import concourse.tile as tile
from concourse import bacc, mybir
from concourse import bass_utils

N_CORES = 8
B = 4096
N = 128
BC = B // N_CORES          # 512 items per core
QUADS = BC // 4            # 128 groups of 4 items
F32 = mybir.dt.float32
BF16 = mybir.dt.bfloat16
AF = mybir.ActivationFunctionType
ALU = mybir.AluOpType
BF = ml_dtypes.bfloat16

_cache = {}


def _build():
    nc = bacc.Bacc(
        "TRN2",
        target_bir_lowering=False,
        debug=False,
        enable_asserts=False,
        num_devices=N_CORES,
    )
    # per-quad data: qt [16, 1024] block-diag A^T s'^T (both branches) then
    # sst [16, 128] = stacked s'^T, packed in one contiguous record
    qtss_t = nc.dram_tensor("qtss", [QUADS, 16, 1152], BF16, kind="ExternalInput")
    # zero-padded s' variants: cols (i, k16); variant i nonzero only in 4i..4i+4
    snatp_t = nc.dram_tensor("snatp", [QUADS, 128, 64], BF16, kind="ExternalInput")
    rep16_t = nc.dram_tensor("rep16", [16, 16], BF16, kind="ExternalInput")
    wcrs_t = nc.dram_tensor("wcrs", [4, 64], F32, kind="ExternalInput")
    wctg_t = nc.dram_tensor("wctg", [4, 64], F32, kind="ExternalInput")
    w1_t = nc.dram_tensor("w1", [64, 128], BF16, kind="ExternalInput")
    w2_t = nc.dram_tensor("w2", [128, 128], BF16, kind="ExternalInput")
    w3_t = nc.dram_tensor("w3", [128, 1], BF16, kind="ExternalInput")
    b1_t = nc.dram_tensor("b1", [128, 1], F32, kind="ExternalInput")
    b2_t = nc.dram_tensor("b2", [128, 1], F32, kind="ExternalInput")
    b3_t = nc.dram_tensor("b3rep", [1, BC], F32, kind="ExternalInput")
    out_t = nc.dram_tensor("out", [BC, 1], F32, kind="ExternalOutput")

    qtss_ap = qtss_t.ap()
    snatp_ap = snatp_t.ap()

    with tile.TileContext(nc) as tc:
        with (
            tc.tile_pool(name="singles", bufs=1) as singles,
            tc.tile_pool(name="qsb", bufs=4) as qsb,
            tc.tile_pool(name="pst", bufs=4, space="PSUM") as pst,
            tc.tile_pool(name="psmall", bufs=3, space="PSUM") as psmall,
        ):
            rep16 = singles.tile([16, 16], BF16)
            nc.sync.dma_start(rep16[:], rep16_t.ap())
            wcrs = singles.tile([4, 64], F32)
            nc.sync.dma_start(wcrs[:], wcrs_t.ap())
            wctg = singles.tile([4, 64], F32)
            nc.sync.dma_start(wctg[:], wctg_t.ap())
            w1 = singles.tile([64, 128], BF16)
            nc.sync.dma_start(w1[:], w1_t.ap())
            w2 = singles.tile([128, 128], BF16)
            nc.sync.dma_start(w2[:], w2_t.ap())
            w3 = singles.tile([128, 1], BF16)
            nc.sync.dma_start(w3[:], w3_t.ap())
            b1 = singles.tile([128, 1], F32)
            nc.sync.dma_start(b1[:], b1_t.ap())
            b2 = singles.tile([128, 1], F32)
            nc.sync.dma_start(b2[:], b2_t.ap())
            b3r = singles.tile([1, BC], F32)
            nc.sync.dma_start(b3r[:], b3_t.ap())
            # t accumulator: rows (i,k), cols (quad, branch)
            tbig = singles.tile([16, 2 * QUADS], F32)

            for q in range(QUADS):
                qtss = qsb.tile([16, 1152], BF16, tag="qtss")
                nc.sync.dma_start(qtss[:], qtss_ap[q])
                snatp = qsb.tile([128, 64], BF16, tag="snatp")
                nc.gpsimd.dma_start(snatp[:], snatp_ap[q])
                sst = qtss[:, 1024:1152]

                # X = S^T per item (key idx on partitions), both branches
                ps_st_rs = pst.tile([128, 512], F32, tag="st")
                ps_st_tg = pst.tile([128, 512], F32, tag="st")
                nc.tensor.matmul(ps_st_rs[:], sst, qtss[:, 0:512])
                nc.tensor.matmul(ps_st_tg[:], sst, qtss[:, 512:1024])

                # U^T = exp(X), bf16; cols (b, i, n)
                ut = qsb.tile([128, 1024], BF16, tag="ut")
                nc.scalar.activation(ut[:, 0:512], ps_st_rs[:], AF.Exp)
                nc.scalar.activation(ut[:, 512:1024], ps_st_tg[:], AF.Exp)

                # G[(i,k), (b,n)] = sum_m s'_i[m,k] U^T[m,(b,n)]; row (i,3)=Z
                psgz = psmall.tile([16, 512], F32, tag="psgz")
                ps_g = psgz[:, 0:256]
                ps_zrep = psgz[:, 256:512]
                ut3 = ut[:].rearrange("m (b i n) -> m b i n", b=2, i=4)
                for i in range(4):
                    nc.tensor.matmul(
                        ps_g,
                        snatp[:, 16 * i:16 * (i + 1)],
                        ut3[:, :, i, :],
                        start=(i == 0),
                        stop=(i == 3),
                    )

                # replicate each item's Z row over its 4 rows, then 1/Z
                g_sb = qsb.tile([16, 256], BF16, tag="g_sb")
                nc.vector.tensor_copy(g_sb[:], ps_g)
                nc.tensor.matmul(ps_zrep, rep16[:], g_sb[:])
                r_sb = qsb.tile([16, 256], F32, tag="r_sb")
                nc.vector.reciprocal_approx_fast(r_sb[:], ps_zrep)

                # t[(i,k), (q,b)] = sum_n G * (1/Z)
                pg = qsb.tile([16, 256], F32, tag="pg")
                nc.gpsimd.tensor_mul(pg[:], g_sb[:], r_sb[:])
                pg3 = pg[:].rearrange("p (b n) -> p b n", b=2)
                nc.vector.tensor_reduce(
                    tbig[:, 2 * q:2 * (q + 1)], pg3,
                    axis=mybir.AxisListType.X, op=ALU.add,
                )

            # repack t to [4, (b, q, i)] = [4(k), (b, item)] via sbuf-to-sbuf
            # DMA (engine APs cannot start at partition 4, DMA descriptors can)
            trs = singles.tile([4, 2 * BC], F32)
            trs4 = trs[:].rearrange("p (b q i) -> p b q i", b=2, i=4)
            for i in range(4):
                src = tbig[4 * i:4 * (i + 1), :].rearrange("p (q b) -> p b q", b=2)
                for b in range(2):
                    nc.sync.dma_start(trs4[:, b, :, i], src[:, b, :])

            # batched MLP over all BC items
            ps_h = pst.tile([64, BC], F32, tag="st")
            nc.tensor.matmul(ps_h[:], wcrs[:], trs[:, 0:BC], start=True, stop=False)
            nc.tensor.matmul(ps_h[:], wctg[:], trs[:, BC:2 * BC], start=False, stop=True)
            h_sb = singles.tile([64, BC], BF16)
            nc.vector.tensor_copy(h_sb[:], ps_h[:])

            ps_z1 = pst.tile([128, BC], F32, tag="st")
            nc.tensor.matmul(ps_z1[:], w1[:], h_sb[:])
            h1 = singles.tile([128, BC], BF16)
            nc.scalar.activation(h1[:], ps_z1[:], AF.Tanh, bias=b1[:])

            ps_z2 = pst.tile([128, BC], F32, tag="st")
            nc.tensor.matmul(ps_z2[:], w2[:], h1[:])
            h2 = singles.tile([128, BC], BF16)
            nc.scalar.activation(h2[:], ps_z2[:], AF.Tanh, bias=b2[:])

            ps_z3 = psmall.tile([1, BC], F32, tag="psgz")
            nc.tensor.matmul(ps_z3[:], w3[:], h2[:])
            y_sb = singles.tile([1, BC], F32)
            nc.vector.tensor_add(y_sb[:], ps_z3[:], b3r[:])

            nc.sync.dma_start(out_t.ap().rearrange("b o -> o b"), y_sb[:])

    nc.compile()
    return nc


def _host_prep(inputs):
    f = lambda x: np.asarray(x, dtype=np.float32)
    s_obs = f(inputs["s_obs"])

    def aug_w(W, b):
        return np.vstack([f(W), f(b).reshape(1, -1)])  # [4, dout]

    Wq_rs = aug_w(inputs["Wq_rs"], inputs["bq_rs"])
    Wk_rs = aug_w(inputs["Wk_rs"], inputs["bk_rs"])
    Wv_rs = aug_w(inputs["Wv_rs"], inputs["bv_rs"])
    Wq_tg = aug_w(inputs["Wq_tg"], inputs["bq_tg"])
    Wk_tg = aug_w(inputs["Wk_tg"], inputs["bk_tg"])
    Wv_tg = aug_w(inputs["Wv_tg"], inputs["bv_tg"])

    scale = 1.0 / np.sqrt(16.0)
    A_rs = (Wq_rs @ Wk_rs.T * scale).astype(np.float32)   # [4, 4]
    A_tg = (Wq_tg @ Wk_tg.T * scale).astype(np.float32)

    ones = np.ones((B, N, 1), np.float32)
    s_aug = np.concatenate([s_obs, ones], axis=2)          # [B, 128, 4]

    # Y_b[item] = A_b^T s'^T : [B, 4, 128]
    Y = np.stack([
        np.einsum("kj,ink->ijn", A_rs, s_aug),
        np.einsum("kj,ink->ijn", A_tg, s_aug),
    ], axis=0).astype(np.float32)                          # [2, B, 4, 128]

    rep16 = np.zeros((16, 16), BF)
    for i in range(4):
        rep16[4 * i + 3, 4 * i:4 * (i + 1)] = 1.0

    wcrs = np.zeros((4, 64), np.float32)
    wctg = np.zeros((4, 64), np.float32)
    wcrs[:, 0:32] = Wv_rs / N
    wctg[:, 32:64] = Wv_tg / N

    w1 = f(inputs["W1"])                       # [64, 128]
    b1 = f(inputs["b1"]).reshape(128, 1)
    w2 = f(inputs["W2"])                       # [128, 128]
    b2 = f(inputs["b2"]).reshape(128, 1)
    w3 = f(inputs["W3"])                       # [128, 1]
    b3rep = np.full((1, BC), float(np.asarray(inputs["b3"]).reshape(-1)[0]),
                    np.float32)

    common = dict(
        rep16=rep16,
        wcrs=wcrs, wctg=wctg,
        w1=w1.astype(BF), w2=w2.astype(BF), w3=w3.astype(BF),
        b1=b1, b2=b2, b3rep=b3rep,
    )

    in_maps = []
    for c in range(N_CORES):
        lo, hi = c * BC, (c + 1) * BC
        sa = s_aug[lo:hi].reshape(QUADS, 4, N, 4)          # [Q, i, n, k]
        Yc = Y[:, lo:hi].reshape(2, QUADS, 4, 4, N)        # [b, Q, i, j, n]

        # qt [Q, (i,j)=16, (b,i',n)=1024], block-diagonal in (i, i')
        qt = np.zeros((QUADS, 4, 4, 2, 4, N), np.float32)  # q i j b i' n
        for i in range(4):
            qt[:, i, :, 0, i, :] = Yc[0, :, i]
            qt[:, i, :, 1, i, :] = Yc[1, :, i]
        qt = qt.reshape(QUADS, 16, 1024)

        # sst [Q, (i,k)=16, n=128]
        sst = sa.transpose(0, 1, 3, 2).reshape(QUADS, 16, N)

        qtss = np.concatenate([qt, sst], axis=2).astype(BF)  # [Q, 16, 1152]

        # snatp [Q, 128, (i, k16)=64]: variant i nonzero only in cols 4i..4i+4
        snatp = np.zeros((QUADS, N, 4, 16), np.float32)
        for i in range(4):
            snatp[:, :, i, 4 * i:4 * (i + 1)] = sa[:, i]
        snatp = snatp.reshape(QUADS, N, 64).astype(BF)

        m = dict(common)
        m["qtss"] = np.ascontiguousarray(qtss)
        m["snatp"] = np.ascontiguousarray(snatp)
        in_maps.append(m)
    return in_maps


def kernel(**inputs):
    if "nc" not in _cache:
        _cache["nc"] = _build()
    nc = _cache["nc"]
    in_maps = _host_prep(inputs)
    trace = os.environ.get("KERNEL_TRACE", "0") == "1"
    res = bass_utils.run_bass_kernel_spmd(
        nc, in_maps, core_ids=list(range(N_CORES)), trace=trace
    )
    _cache["last"] = res
    out = np.concatenate([r["out"] for r in res.results], axis=0)
    return out.astype(np.float32)


# revision 18
# speedup vs baseline: 5.4022x; 1.1748x over previous
"""Trainium2 Bass kernel for nn_CriticUAVob (attention-pool critic).

Math per item b (4096 total): two attention-pool branches over s_b [N=128, 3]
followed by a small MLP.  With s' = [s, 1] (N x 4) and A = Wq' Wk'^T / 4:

    S = s' A s'^T,  U = exp(S),  Z[n] = sum_m U[n,m]
    pooled = (1/N) * sum_n (U[n,:] / Z[n]) @ V,   V = s' Wv'
           = (1/N) * t^T Wv',   t[k] = sum_n (sum_m U[n,m] s'[m,k]) / Z[n]

Device pipeline (per quad of 4 items, batch data-parallel over 8 cores):
  - one DMA brings Y = A^T s'^T per item in a block-diagonal layout (qt) plus
    s'^T stacked (sst); a second DMA brings zero-padded s' variants (snatp)
  - 2 matmuls (lhsT=sst[16,128], rhs=qt[16,512]) -> X = S^T per item, both
    branches; the block-diagonal qt kills cross-item terms
  - 2 ScalarE exp -> U^T in bf16
  - 4 accumulating matmuls (lhsT = zero-padded s'_i [128,16], rhs = U^T item
    cols) -> G[(i,k),(b,n)] with no cross-item garbage; s' ones-column makes
    row (i,3) = Z
  - tiny const matmul replicates each Z row over its item's 4 rows; vector
    reciprocal + 2 fused multiply-reduce (tensor_tensor_reduce) produce
    t[(i,k)] per branch straight into an accumulator tile
  - batched MLP over all 512 items at the end

All PE inputs are bf16 (4x matmul throughput vs fp32, half the LDWEIGHTS).
"""
import os
import sys
import numpy as np
import ml_dtypes

sys.path.insert(0, "/opt/trn_rl_repo")

import concourse.bass as bass
import concourse.tile as tile
from concourse import bacc, mybir
from concourse import bass_utils

N_CORES = 8
B = 4096
N = 128
BC = B // N_CORES          # 512 items per core
QUADS = BC // 4            # 128 groups of 4 items
F32 = mybir.dt.float32
BF16 = mybir.dt.bfloat16
AF = mybir.ActivationFunctionType
ALU = mybir.AluOpType
BF = ml_dtypes.bfloat16

_cache = {}


def _build():
    nc = bacc.Bacc(
        "TRN2",
        target_bir_lowering=False,
        debug=False,
        enable_asserts=False,
        num_devices=N_CORES,
    )
    # per-quad data: qt [16, 1024] block-diag A^T s'^T (both branches) then
    # sst [16, 128] = stacked s'^T, packed in one contiguous record
    qtss_t = nc.dram_tensor("qtss", [QUADS, 16, 1152], BF16, kind="ExternalInput")
    # zero-padded s' variants: cols (i, k16); variant i nonzero only in 4i..4i+4
    snatp_t = nc.dram_tensor("snatp", [QUADS, 128, 64], BF16, kind="ExternalInput")
    rep16_t = nc.dram_tensor("rep16", [16, 16], BF16, kind="ExternalInput")
    wcrs_t = nc.dram_tensor("wcrs", [4, 64], F32, kind="ExternalInput")
    wctg_t = nc.dram_tensor("wctg", [4, 64], F32, kind="ExternalInput")
    w1_t = nc.dram_tensor("w1", [64, 128], BF16, kind="ExternalInput")
    w2_t = nc.dram_tensor("w2", [128, 128], BF16, kind="ExternalInput")
    w3_t = nc.dram_tensor("w3", [128, 1], BF16, kind="ExternalInput")
    b1_t = nc.dram_tensor("b1", [128, 1], F32, kind="ExternalInput")
    b2_t = nc.dram_tensor("b2", [128, 1], F32, kind="ExternalInput")
    b3_t = nc.dram_tensor("b3rep", [1, BC], F32, kind="ExternalInput")
    out_t = nc.dram_tensor("out", [BC, 1], F32, kind="ExternalOutput")

    qtss_ap = qtss_t.ap()
    snatp_ap = snatp_t.ap()

    with tile.TileContext(nc) as tc:
        with (
            tc.tile_pool(name="singles", bufs=1) as singles,
            tc.tile_pool(name="qsb", bufs=4) as qsb,
            tc.tile_pool(name="pst", bufs=2, space="PSUM") as pst,
            tc.tile_pool(name="psmall", bufs=3, space="PSUM") as psmall,
        ):
            rep16 = singles.tile([16, 16], BF16)
            nc.sync.dma_start(rep16[:], rep16_t.ap())
            wcrs = singles.tile([4, 64], F32)
            nc.sync.dma_start(wcrs[:], wcrs_t.ap())
            wctg = singles.tile([4, 64], F32)
            nc.sync.dma_start(wctg[:], wctg_t.ap())
            w1 = singles.tile([64, 128], BF16)
            nc.sync.dma_start(w1[:], w1_t.ap())
            w2 = singles.tile([128, 128], BF16)
            nc.sync.dma_start(w2[:], w2_t.ap())
            w3 = singles.tile([128, 1], BF16)
            nc.sync.dma_start(w3[:], w3_t.ap())
            b1 = singles.tile([128, 1], F32)
            nc.sync.dma_start(b1[:], b1_t.ap())
            b2 = singles.tile([128, 1], F32)
            nc.sync.dma_start(b2[:], b2_t.ap())
            b3r = singles.tile([1, BC], F32)
            nc.sync.dma_start(b3r[:], b3_t.ap())
            # t accumulator: rows (i,k), cols (quad, branch)
            tbig = singles.tile([16, 2 * QUADS], F32)

            # Software pipeline: stage lags keep every engine's next
            # instruction dependent only on work from >=1 iteration ago, so
            # the PE never stalls mid-stream (and can ramp to full clock).
            qtssT, snatpT, ps_stT, utT, psgzT, g_sbT, r_sbT, pgT = (
                {}, {}, {}, {}, {}, {}, {}, {},
            )
            L_DMA, L_ST, L_G, L_Z, L_T = 0, 1, 2, 3, 4

            def live(j, lag):
                return 0 <= j - lag < QUADS

            for j in range(QUADS + L_T + 1):
                if live(j, L_DMA):
                    q = j
                    qtssT[q] = qsb.tile([16, 1152], BF16, tag="qtss", name="qtss")
                    nc.sync.dma_start(qtssT[q][:], qtss_ap[q])
                    snatpT[q] = qsb.tile([128, 64], BF16, tag="snatp", name="snatp")
                    nc.gpsimd.dma_start(snatpT[q][:], snatp_ap[q])

                if live(j, L_ST):
                    # X = S^T per item (key idx on partitions), both branches
                    q = j - L_ST
                    qtss = qtssT[q]
                    sst = qtss[:, 1024:1152]
                    ps_st = pst.tile([128, 1024], F32, tag="st", name="ps_st")
                    ps_stT[q] = ps_st
                    nc.tensor.matmul(ps_st[:, 0:512], sst, qtss[:, 0:512])
                    nc.tensor.matmul(ps_st[:, 512:1024], sst, qtss[:, 512:1024])
                    # U^T = exp(X), bf16; cols (b, i, n)
                    ut = qsb.tile([128, 1024], BF16, tag="ut", name="ut")
                    utT[q] = ut
                    nc.scalar.activation(ut[:], ps_st[:], AF.Exp)
                    del qtssT[q]

                if live(j, L_G):
                    # G[(i,k),(b,n)] = sum_m s'_i[m,k] U^T[m,(b,n)]; (i,3)=Z
                    q = j - L_G
                    psgz = psmall.tile([16, 512], F32, tag="psgz", name="psgz")
                    psgzT[q] = psgz
                    ut3 = utT[q][:].rearrange("m (b i n) -> m b i n", b=2, i=4)
                    for i in range(4):
                        nc.tensor.matmul(
                            psgz[:, 0:256],
                            snatpT[q][:, 16 * i:16 * (i + 1)],
                            ut3[:, :, i, :],
                            start=(i == 0),
                            stop=(i == 3),
                        )
                    g_sb = qsb.tile([16, 256], BF16, tag="g_sb", name="g_sb")
                    g_sbT[q] = g_sb
                    nc.vector.tensor_copy(g_sb[:], psgz[:, 0:256])
                    del ps_stT[q], snatpT[q]

                if live(j, L_Z):
                    # replicate each item's Z row over its 4 rows, then 1/Z
                    q = j - L_Z
                    psgz = psgzT[q]
                    nc.tensor.matmul(psgz[:, 256:512], rep16[:], g_sbT[q][:])
                    r_sb = qsb.tile([16, 256], F32, tag="r_sb", name="r_sb")
                    r_sbT[q] = r_sb
                    nc.vector.reciprocal_approx_fast(r_sb[:], psgz[:, 256:512])
                    del utT[q]

                if live(j, L_T):
                    # t[(i,k), (q,b)] = sum_n G * (1/Z)
                    q = j - L_T
                    pg = qsb.tile([16, 256], F32, tag="pg", name="pg")
                    nc.gpsimd.tensor_mul(pg[:], g_sbT[q][:], r_sbT[q][:])
                    pg3 = pg[:].rearrange("p (b n) -> p b n", b=2)
                    nc.vector.tensor_reduce(
                        tbig[:, 2 * q:2 * (q + 1)], pg3,
                        axis=mybir.AxisListType.X, op=ALU.add,
                    )
                    del psgzT[q], g_sbT[q], r_sbT[q]

            # repack t to [4, (b, q, i)] = [4(k), (b, item)] via sbuf-to-sbuf
            # DMA (engine APs cannot start at partition 4, DMA descriptors can)
            trs = singles.tile([4, 2 * BC], F32)
            trs4 = trs[:].rearrange("p (b q i) -> p b q i", b=2, i=4)
            for i in range(4):
                src = tbig[4 * i:4 * (i + 1), :].rearrange("p (q b) -> p b q", b=2)
                for b in range(2):
                    nc.sync.dma_start(trs4[:, b, :, i], src[:, b, :])

            # batched MLP over all BC items
            ps_h = pst.tile([64, BC], F32, tag="st")
            nc.tensor.matmul(ps_h[:], wcrs[:], trs[:, 0:BC], start=True, stop=False)
            nc.tensor.matmul(ps_h[:], wctg[:], trs[:, BC:2 * BC], start=False, stop=True)
            h_sb = singles.tile([64, BC], BF16)
            nc.vector.tensor_copy(h_sb[:], ps_h[:])

            ps_z1 = pst.tile([128, BC], F32, tag="st")
            nc.tensor.matmul(ps_z1[:], w1[:], h_sb[:])
            h1 = singles.tile([128, BC], BF16)
            nc.scalar.activation(h1[:], ps_z1[:], AF.Tanh, bias=b1[:])

            ps_z2 = pst.tile([128, BC], F32, tag="st")
            nc.tensor.matmul(ps_z2[:], w2[:], h1[:])
            h2 = singles.tile([128, BC], BF16)
            nc.scalar.activation(h2[:], ps_z2[:], AF.Tanh, bias=b2[:])

            ps_z3 = psmall.tile([1, BC], F32, tag="psgz")
            nc.tensor.matmul(ps_z3[:], w3[:], h2[:])
            y_sb = singles.tile([1, BC], F32)
            nc.vector.tensor_add(y_sb[:], ps_z3[:], b3r[:])

            nc.sync.dma_start(out_t.ap().rearrange("b o -> o b"), y_sb[:])

    nc.compile()
    return nc


def _host_prep(inputs):
    f = lambda x: np.asarray(x, dtype=np.float32)
    s_obs = f(inputs["s_obs"])

    def aug_w(W, b):
        return np.vstack([f(W), f(b).reshape(1, -1)])  # [4, dout]

    Wq_rs = aug_w(inputs["Wq_rs"], inputs["bq_rs"])
    Wk_rs = aug_w(inputs["Wk_rs"], inputs["bk_rs"])
    Wv_rs = aug_w(inputs["Wv_rs"], inputs["bv_rs"])
    Wq_tg = aug_w(inputs["Wq_tg"], inputs["bq_tg"])
    Wk_tg = aug_w(inputs["Wk_tg"], inputs["bk_tg"])
    Wv_tg = aug_w(inputs["Wv_tg"], inputs["bv_tg"])

    scale = 1.0 / np.sqrt(16.0)
    A_rs = (Wq_rs @ Wk_rs.T * scale).astype(np.float32)   # [4, 4]
    A_tg = (Wq_tg @ Wk_tg.T * scale).astype(np.float32)

    ones = np.ones((B, N, 1), np.float32)
    s_aug = np.concatenate([s_obs, ones], axis=2)          # [B, 128, 4]

    # Y_b[item] = A_b^T s'^T : [B, 4, 128]
    Y = np.stack([
        np.einsum("kj,ink->ijn", A_rs, s_aug),
        np.einsum("kj,ink->ijn", A_tg, s_aug),
    ], axis=0).astype(np.float32)                          # [2, B, 4, 128]

    rep16 = np.zeros((16, 16), BF)
    for i in range(4):
        rep16[4 * i + 3, 4 * i:4 * (i + 1)] = 1.0

    wcrs = np.zeros((4, 64), np.float32)
    wctg = np.zeros((4, 64), np.float32)
    wcrs[:, 0:32] = Wv_rs / N
    wctg[:, 32:64] = Wv_tg / N

    w1 = f(inputs["W1"])                       # [64, 128]
    b1 = f(inputs["b1"]).reshape(128, 1)
    w2 = f(inputs["W2"])                       # [128, 128]
    b2 = f(inputs["b2"]).reshape(128, 1)
    w3 = f(inputs["W3"])                       # [128, 1]
    b3rep = np.full((1, BC), float(np.asarray(inputs["b3"]).reshape(-1)[0]),
                    np.float32)

    common = dict(
        rep16=rep16,
        wcrs=wcrs, wctg=wctg,
        w1=w1.astype(BF), w2=w2.astype(BF), w3=w3.astype(BF),
        b1=b1, b2=b2, b3rep=b3rep,
    )

    in_maps = []
    for c in range(N_CORES):
        lo, hi = c * BC, (c + 1) * BC
        sa = s_aug[lo:hi].reshape(QUADS, 4, N, 4)          # [Q, i, n, k]
        Yc = Y[:, lo:hi].reshape(2, QUADS, 4, 4, N)        # [b, Q, i, j, n]

        # qt [Q, (i,j)=16, (b,i',n)=1024], block-diagonal in (i, i')
        qt = np.zeros((QUADS, 4, 4, 2, 4, N), np.float32)  # q i j b i' n
        for i in range(4):
            qt[:, i, :, 0, i, :] = Yc[0, :, i]
            qt[:, i, :, 1, i, :] = Yc[1, :, i]
        qt = qt.reshape(QUADS, 16, 1024)

        # sst [Q, (i,k)=16, n=128]
        sst = sa.transpose(0, 1, 3, 2).reshape(QUADS, 16, N)

        qtss = np.concatenate([qt, sst], axis=2).astype(BF)  # [Q, 16, 1152]

        # snatp [Q, 128, (i, k16)=64]: variant i nonzero only in cols 4i..4i+4
        snatp = np.zeros((QUADS, N, 4, 16), np.float32)
        for i in range(4):
            snatp[:, :, i, 4 * i:4 * (i + 1)] = sa[:, i]
        snatp = snatp.reshape(QUADS, N, 64).astype(BF)

        m = dict(common)
        m["qtss"] = np.ascontiguousarray(qtss)
        m["snatp"] = np.ascontiguousarray(snatp)
        in_maps.append(m)
    return in_maps


def kernel(**inputs):
    if "nc" not in _cache:
        _cache["nc"] = _build()
    nc = _cache["nc"]
    in_maps = _host_prep(inputs)
    trace = os.environ.get("KERNEL_TRACE", "0") == "1"
    res = bass_utils.run_bass_kernel_spmd(
        nc, in_maps, core_ids=list(range(N_CORES)), trace=trace
    )
    _cache["last"] = res
    out = np.concatenate([r["out"] for r in res.results], axis=0)
    return out.astype(np.float32)


# revision 23
# speedup vs baseline: 5.6657x; 1.0488x over previous
"""Trainium2 Bass kernel for nn_CriticUAVob (attention-pool critic).

Math per item b (4096 total): two attention-pool branches over s_b [N=128, 3]
followed by a small MLP.  With s' = [s, 1] (N x 4) and A = Wq' Wk'^T / 4:

    S = s' A s'^T,  U = exp(S),  Z[n] = sum_m U[n,m]
    pooled = (1/N) * sum_n (U[n,:] / Z[n]) @ V,   V = s' Wv'
           = (1/N) * t^T Wv',   t[k] = sum_n (sum_m U[n,m] s'[m,k]) / Z[n]

Device pipeline (per quad of 4 items, batch data-parallel over 8 cores):
  - one DMA brings Y = A^T s'^T per item in a block-diagonal layout (qt) plus
    s'^T stacked (sst); a second DMA brings zero-padded s' variants (snatp)
  - 2 matmuls (lhsT=sst[16,128], rhs=qt[16,512]) -> X = S^T per item, both
    branches; the block-diagonal qt kills cross-item terms
  - 2 ScalarE exp -> U^T in bf16
  - 4 accumulating matmuls (lhsT = zero-padded s'_i [128,16], rhs = U^T item
    cols) -> G[(i,k),(b,n)] with no cross-item garbage; s' ones-column makes
    row (i,3) = Z
  - tiny const matmul replicates each Z row over its item's 4 rows; vector
    reciprocal + 2 fused multiply-reduce (tensor_tensor_reduce) produce
    t[(i,k)] per branch straight into an accumulator tile
  - batched MLP over all 512 items at the end

All PE inputs are bf16 (4x matmul throughput vs fp32, half the LDWEIGHTS).
"""
import os
import sys
import numpy as np
import ml_dtypes

sys.path.insert(0, "/opt/trn_rl_repo")

import concourse.bass as bass
import concourse.tile as tile
from concourse import bacc, mybir
from concourse import bass_utils

N_CORES = 8
B = 4096
N = 128
BC = B // N_CORES          # 512 items per core
QUADS = BC // 4            # 128 groups of 4 items
F32 = mybir.dt.float32
BF16 = mybir.dt.bfloat16
AF = mybir.ActivationFunctionType
ALU = mybir.AluOpType
BF = ml_dtypes.bfloat16

_cache = {}


def _build():
    nc = bacc.Bacc(
        "TRN2",
        target_bir_lowering=False,
        debug=False,
        enable_asserts=False,
        num_devices=N_CORES,
    )
    # per-quad data: qt [16, 1024] block-diag A^T s'^T (both branches) then
    # sst [16, 128] = stacked s'^T, packed in one contiguous record
    qtss_t = nc.dram_tensor("qtss", [QUADS, 16, 1152], BF16, kind="ExternalInput")
    # zero-padded s' variants: cols (i, k16); variant i nonzero only in 4i..4i+4
    snatp_t = nc.dram_tensor("snatp", [QUADS, 128, 64], BF16, kind="ExternalInput")
    rep16_t = nc.dram_tensor("rep16", [16, 16], BF16, kind="ExternalInput")
    wcrs_t = nc.dram_tensor("wcrs", [4, 64], F32, kind="ExternalInput")
    wctg_t = nc.dram_tensor("wctg", [4, 64], F32, kind="ExternalInput")
    w1_t = nc.dram_tensor("w1", [64, 128], BF16, kind="ExternalInput")
    w2_t = nc.dram_tensor("w2", [128, 128], BF16, kind="ExternalInput")
    w3_t = nc.dram_tensor("w3", [128, 1], BF16, kind="ExternalInput")
    b1_t = nc.dram_tensor("b1", [128, 1], F32, kind="ExternalInput")
    b2_t = nc.dram_tensor("b2", [128, 1], F32, kind="ExternalInput")
    b3_t = nc.dram_tensor("b3rep", [1, BC], F32, kind="ExternalInput")
    out_t = nc.dram_tensor("out", [BC, 1], F32, kind="ExternalOutput")

    qtss_ap = qtss_t.ap()
    snatp_ap = snatp_t.ap()

    with tile.TileContext(nc) as tc:
        with (
            tc.tile_pool(name="singles", bufs=1) as singles,
            tc.tile_pool(name="qsb", bufs=4) as qsb,
            tc.tile_pool(name="pst", bufs=3, space="PSUM") as pst,
            tc.tile_pool(name="psmall", bufs=2, space="PSUM") as psmall,
        ):
            rep16 = singles.tile([16, 16], BF16)
            nc.sync.dma_start(rep16[:], rep16_t.ap())
            wcrs = singles.tile([4, 64], F32)
            nc.sync.dma_start(wcrs[:], wcrs_t.ap())
            wctg = singles.tile([4, 64], F32)
            nc.sync.dma_start(wctg[:], wctg_t.ap())
            w1 = singles.tile([64, 128], BF16)
            nc.sync.dma_start(w1[:], w1_t.ap())
            w2 = singles.tile([128, 128], BF16)
            nc.sync.dma_start(w2[:], w2_t.ap())
            w3 = singles.tile([128, 1], BF16)
            nc.sync.dma_start(w3[:], w3_t.ap())
            b1 = singles.tile([128, 1], F32)
            nc.sync.dma_start(b1[:], b1_t.ap())
            b2 = singles.tile([128, 1], F32)
            nc.sync.dma_start(b2[:], b2_t.ap())
            b3r = singles.tile([1, BC], F32)
            nc.sync.dma_start(b3r[:], b3_t.ap())
            # t accumulator: rows (i,k), cols (quad, branch)
            tbig = singles.tile([16, 2 * QUADS], F32)

            # Software pipeline: stage lags keep every engine's next
            # instruction dependent only on work from >=1 iteration ago, so
            # the PE never stalls mid-stream (and can ramp to full clock).
            qtssT, snatpT, ps_stT, utT, psgzT, g_sbT, r_sbT, pgT = (
                {}, {}, {}, {}, {}, {}, {}, {},
            )
            L_DMA, L_ST, L_G, L_Z, L_T = 0, 1, 2, 3, 4

            def live(j, lag):
                return 0 <= j - lag < QUADS

            for j in range(QUADS + L_T + 1):
                if live(j, L_DMA):
                    q = j
                    qtssT[q] = qsb.tile([16, 1152], BF16, tag="qtss", name="qtss")
                    nc.sync.dma_start(qtssT[q][:], qtss_ap[q])
                    snatpT[q] = qsb.tile([128, 64], BF16, tag="snatp", name="snatp")
                    nc.gpsimd.dma_start(snatpT[q][:], snatp_ap[q])

                if live(j, L_ST):
                    # X = S^T per item (key idx on partitions), both branches
                    q = j - L_ST
                    qtss = qtssT[q]
                    sst = qtss[:, 1024:1152]
                    ps_st = pst.tile([128, 1024], F32, tag="st", name="ps_st")
                    ps_stT[q] = ps_st
                    nc.tensor.matmul(ps_st[:, 0:512], sst, qtss[:, 0:512])
                    nc.tensor.matmul(ps_st[:, 512:1024], sst, qtss[:, 512:1024])
                    # U^T = exp(X), bf16; stored as cols (i, b, n) so each
                    # item's G-matmul rhs is a contiguous 2D slice
                    ut = qsb.tile([128, 1024], BF16, tag="ut", name="ut")
                    utT[q] = ut
                    ut_v = ut[:].rearrange("m (i b n) -> m b i n", i=4, b=2)
                    ps_st_v = ps_st[:].rearrange("m (b i n) -> m b i n", b=2, i=4)
                    nc.scalar.activation(ut_v, ps_st_v, AF.Exp)
                    del qtssT[q]

                if live(j, L_G):
                    # G[(i,k),(b,n)] = sum_m s'_i[m,k] U^T[m,(b,n)]; (i,3)=Z
                    q = j - L_G
                    psgz = psmall.tile([16, 512], F32, tag="psgz", name="psgz")
                    psgzT[q] = psgz
                    for i in range(4):
                        nc.tensor.matmul(
                            psgz[:, 0:256],
                            snatpT[q][:, 16 * i:16 * (i + 1)],
                            utT[q][:, 256 * i:256 * (i + 1)],
                            start=(i == 0),
                            stop=(i == 3),
                        )
                    g_sb = qsb.tile([16, 256], BF16, tag="g_sb", name="g_sb")
                    g_sbT[q] = g_sb
                    nc.vector.tensor_copy(g_sb[:], psgz[:, 0:256])
                    del ps_stT[q], snatpT[q]

                if live(j, L_Z):
                    # replicate each item's Z row over its 4 rows, then 1/Z
                    q = j - L_Z
                    psgz = psgzT[q]
                    nc.tensor.matmul(psgz[:, 256:512], rep16[:], g_sbT[q][:])
                    r_sb = qsb.tile([16, 256], F32, tag="r_sb", name="r_sb")
                    r_sbT[q] = r_sb
                    nc.vector.reciprocal_approx_fast(r_sb[:], psgz[:, 256:512])
                    del utT[q]

                if live(j, L_T):
                    # t[(i,k), (q,b)] = sum_n G * (1/Z)
                    q = j - L_T
                    pg = qsb.tile([16, 256], F32, tag="pg", name="pg")
                    nc.gpsimd.tensor_mul(pg[:], g_sbT[q][:], r_sbT[q][:])
                    pg3 = pg[:].rearrange("p (b n) -> p b n", b=2)
                    nc.vector.tensor_reduce(
                        tbig[:, 2 * q:2 * (q + 1)], pg3,
                        axis=mybir.AxisListType.X, op=ALU.add,
                    )
                    del psgzT[q], g_sbT[q], r_sbT[q]

            # repack t to [4, (b, q, i)] = [4(k), (b, item)] via sbuf-to-sbuf
            # DMA (engine APs cannot start at partition 4, DMA descriptors can)
            trs = singles.tile([4, 2 * BC], F32)
            trs4 = trs[:].rearrange("p (b q i) -> p b q i", b=2, i=4)
            for i in range(4):
                src = tbig[4 * i:4 * (i + 1), :].rearrange("p (q b) -> p b q", b=2)
                for b in range(2):
                    nc.sync.dma_start(trs4[:, b, :, i], src[:, b, :])

            # batched MLP over all BC items
            ps_h = pst.tile([64, BC], F32, tag="st")
            nc.tensor.matmul(ps_h[:], wcrs[:], trs[:, 0:BC], start=True, stop=False)
            nc.tensor.matmul(ps_h[:], wctg[:], trs[:, BC:2 * BC], start=False, stop=True)
            h_sb = singles.tile([64, BC], BF16)
            nc.vector.tensor_copy(h_sb[:], ps_h[:])

            ps_z1 = pst.tile([128, BC], F32, tag="st")
            nc.tensor.matmul(ps_z1[:], w1[:], h_sb[:])
            h1 = singles.tile([128, BC], BF16)
            nc.scalar.activation(h1[:], ps_z1[:], AF.Tanh, bias=b1[:])

            ps_z2 = pst.tile([128, BC], F32, tag="st")
            nc.tensor.matmul(ps_z2[:], w2[:], h1[:])
            h2 = singles.tile([128, BC], BF16)
            nc.scalar.activation(h2[:], ps_z2[:], AF.Tanh, bias=b2[:])

            ps_z3 = psmall.tile([1, BC], F32, tag="psgz")
            nc.tensor.matmul(ps_z3[:], w3[:], h2[:])
            y_sb = singles.tile([1, BC], F32)
            nc.vector.tensor_add(y_sb[:], ps_z3[:], b3r[:])

            nc.sync.dma_start(out_t.ap().rearrange("b o -> o b"), y_sb[:])

    nc.compile()
    return nc


def _host_prep(inputs):
    f = lambda x: np.asarray(x, dtype=np.float32)
    s_obs = f(inputs["s_obs"])

    def aug_w(W, b):
        return np.vstack([f(W), f(b).reshape(1, -1)])  # [4, dout]

    Wq_rs = aug_w(inputs["Wq_rs"], inputs["bq_rs"])
    Wk_rs = aug_w(inputs["Wk_rs"], inputs["bk_rs"])
    Wv_rs = aug_w(inputs["Wv_rs"], inputs["bv_rs"])
    Wq_tg = aug_w(inputs["Wq_tg"], inputs["bq_tg"])
    Wk_tg = aug_w(inputs["Wk_tg"], inputs["bk_tg"])
    Wv_tg = aug_w(inputs["Wv_tg"], inputs["bv_tg"])

    scale = 1.0 / np.sqrt(16.0)
    A_rs = (Wq_rs @ Wk_rs.T * scale).astype(np.float32)   # [4, 4]
    A_tg = (Wq_tg @ Wk_tg.T * scale).astype(np.float32)

    ones = np.ones((B, N, 1), np.float32)
    s_aug = np.concatenate([s_obs, ones], axis=2)          # [B, 128, 4]

    # Y_b[item] = A_b^T s'^T : [B, 4, 128]
    Y = np.stack([
        np.einsum("kj,ink->ijn", A_rs, s_aug),
        np.einsum("kj,ink->ijn", A_tg, s_aug),
    ], axis=0).astype(np.float32)                          # [2, B, 4, 128]

    rep16 = np.zeros((16, 16), BF)
    for i in range(4):
        rep16[4 * i + 3, 4 * i:4 * (i + 1)] = 1.0

    wcrs = np.zeros((4, 64), np.float32)
    wctg = np.zeros((4, 64), np.float32)
    wcrs[:, 0:32] = Wv_rs / N
    wctg[:, 32:64] = Wv_tg / N

    w1 = f(inputs["W1"])                       # [64, 128]
    b1 = f(inputs["b1"]).reshape(128, 1)
    w2 = f(inputs["W2"])                       # [128, 128]
    b2 = f(inputs["b2"]).reshape(128, 1)
    w3 = f(inputs["W3"])                       # [128, 1]
    b3rep = np.full((1, BC), float(np.asarray(inputs["b3"]).reshape(-1)[0]),
                    np.float32)

    common = dict(
        rep16=rep16,
        wcrs=wcrs, wctg=wctg,
        w1=w1.astype(BF), w2=w2.astype(BF), w3=w3.astype(BF),
        b1=b1, b2=b2, b3rep=b3rep,
    )

    in_maps = []
    for c in range(N_CORES):
        lo, hi = c * BC, (c + 1) * BC
        sa = s_aug[lo:hi].reshape(QUADS, 4, N, 4)          # [Q, i, n, k]
        Yc = Y[:, lo:hi].reshape(2, QUADS, 4, 4, N)        # [b, Q, i, j, n]

        # qt [Q, (i,j)=16, (b,i',n)=1024], block-diagonal in (i, i')
        qt = np.zeros((QUADS, 4, 4, 2, 4, N), np.float32)  # q i j b i' n
        for i in range(4):
            qt[:, i, :, 0, i, :] = Yc[0, :, i]
            qt[:, i, :, 1, i, :] = Yc[1, :, i]
        qt = qt.reshape(QUADS, 16, 1024)

        # sst [Q, (i,k)=16, n=128]
        sst = sa.transpose(0, 1, 3, 2).reshape(QUADS, 16, N)

        qtss = np.concatenate([qt, sst], axis=2).astype(BF)  # [Q, 16, 1152]

        # snatp [Q, 128, (i, k16)=64]: variant i nonzero only in cols 4i..4i+4
        snatp = np.zeros((QUADS, N, 4, 16), np.float32)
        for i in range(4):
            snatp[:, :, i, 4 * i:4 * (i + 1)] = sa[:, i]
        snatp = snatp.reshape(QUADS, N, 64).astype(BF)

        m = dict(common)
        m["qtss"] = np.ascontiguousarray(qtss)
        m["snatp"] = np.ascontiguousarray(snatp)
        in_maps.append(m)
    return in_maps


def kernel(**inputs):
    if "nc" not in _cache:
        _cache["nc"] = _build()
    nc = _cache["nc"]
    in_maps = _host_prep(inputs)
    trace = os.environ.get("KERNEL_TRACE", "0") == "1"
    res = bass_utils.run_bass_kernel_spmd(
        nc, in_maps, core_ids=list(range(N_CORES)), trace=trace
    )
    _cache["last"] = res
    out = np.concatenate([r["out"] for r in res.results], axis=0)
    return out.astype(np.float32)
